# revision 7
# baseline (speedup 1.0000x reference)
"""Trainium2 Bass kernel for nn_GaddyMultiheadAttention.

Full-input contract: kernel(**inputs) takes the unsharded tensors
  x (4,2048,768) f32, w_q/w_k/w_v (8,768,96), w_o (8,96,768),
  rel_emb (8,199,96,1)
and returns the full (4,2048,768) f32 output.

Sharding: 8 cores = 4 batches x 2 query-chunks of 1024. No collectives.

Per-core kernel (SPMD, one program):
  - host prep: xT = x[b].T in bf16; weights reshaped to (e, h*d)/(h*d, e)
    bf16 (w_q pre-scaled by d^-0.5); relative-position bias Toeplitz values
    exp(t) reversed+shifted per core ("etre", (8, 3072) bf16).
  - projections: K_h (d,k) d-major, Q_h (d,q) d-major, V (k, h*d interleaved
    with a ones column per head) k-major; all via bf16 matmuls.
  - attention per head, transposed-logits layout: logits_T (128k x 1024q)
    = K_tile^T Q in PSUM, exp on ScalarE, multiply by the exp(bias) strip
    (diagonal sliding-window DMA of etre) on VectorE, then ctx_T (97 x 1024)
    += [V_tile | 1]^T attn on TensorE; ones column yields the softmax
    denominator; normalize via reciprocal + ones-outer-product broadcast.
  - output projection from the stacked normalized ctx (h*d x q) with
    wo (h*d, e), DMA out (1024, 768) f32.
"""

import numpy as np
import ml_dtypes

import concourse.bacc as bacc
import concourse.mybir as mybir
import concourse.tile as tile
from concourse import bass_utils
from bass_rust import VecI64Pair

B, S, E, H, D = 4, 2048, 768, 8, 96
MAX_DIST = 100
QCH = 1024                      # queries per core
NCORES = 8
EC = E // 128                   # 6 contraction chunks of 128
KT = S // 128                   # 16 key tiles
BF16 = mybir.dt.bfloat16
F32 = mybir.dt.float32
NPBF16 = ml_dtypes.bfloat16

_NC_CACHE = {}


def _strip_src_ap(etre_ap, h):
    """AP reading etre[h, p + g] for p in [0,128), g in [0,2944).

    Keys are processed in REVERSED sequence order (host reverses the key
    axis of xT, so K/V columns/rows are key 2047-j').  Then for key-tile kt,
    strip row p = key 2047-(128*kt+p), and
    exp(bias)[key, query=q0+f] == strip[p, 128*kt + f].
    All-positive unit steps — walrus rejects negative partition steps.
    """
    a = etre_ap.copy()
    a.offset = a.offset + 3072 * h
    a.ap = VecI64Pair([[1, 128], [1, 2944]])
    return a


def build_nc():
    nc = bacc.Bacc("TRN2", target_bir_lowering=False, debug=False,
                   num_devices=NCORES)
    xT = nc.dram_tensor("xT", (E, S), BF16, kind="ExternalInput").ap()
    xTq = nc.dram_tensor("xTq", (E, QCH), BF16, kind="ExternalInput").ap()
    wq = nc.dram_tensor("wq", (E, H * D), BF16, kind="ExternalInput").ap()
    wk = nc.dram_tensor("wk", (E, H * D), BF16, kind="ExternalInput").ap()
    wv = nc.dram_tensor("wv", (E, H * D), BF16, kind="ExternalInput").ap()
    wo = nc.dram_tensor("wo", (H * D, E), BF16, kind="ExternalInput").ap()
    etre = nc.dram_tensor("etre", (H, 3072), BF16, kind="ExternalInput").ap()
    out = nc.dram_tensor("out", (QCH, E), F32, kind="ExternalOutput").ap()

    Exp = mybir.ActivationFunctionType.Exp

    with tile.TileContext(nc) as tc:
        with (
            tc.tile_pool(name="pers", bufs=1) as pers,
            tc.tile_pool(name="stripp", bufs=2) as stripp,
            tc.tile_pool(name="attnp", bufs=3) as attnp,
            tc.tile_pool(name="smallp", bufs=2) as smallp,
            tc.tile_pool(name="osbp", bufs=2) as osbp,
            tc.tile_pool(name="psp", bufs=2, space="PSUM") as psp,
            tc.tile_pool(name="psctxp", bufs=2, space="PSUM") as psctxp,
        ):
            # persistent SBUF tiles
            xt = [pers.tile([128, S], BF16, tag=f"xt{c}", name=f"xt{c}") for c in range(EC)]
            xtq = [pers.tile([128, QCH], BF16, tag=f"xtq{c}", name=f"xtq{c}") for c in range(EC)]
            wqs = [pers.tile([128, H * D], BF16, tag=f"wq{c}", name=f"wqs{c}") for c in range(EC)]
            wks = [pers.tile([128, H * D], BF16, tag=f"wk{c}", name=f"wks{c}") for c in range(EC)]
            wvs = [pers.tile([128, H * D], BF16, tag=f"wv{c}", name=f"wvs{c}") for c in range(EC)]
            wos = [pers.tile([128, E], BF16, tag=f"wo{c}", name=f"wos{c}") for c in range(EC)]
            Ksb = [pers.tile([96, S], BF16, tag=f"K{h}", name=f"Ksb{h}") for h in range(H)]
            Qsb = [pers.tile([96, QCH], BF16, tag=f"Q{h}", name=f"Qsb{h}") for h in range(H)]
            Vst = [pers.tile([128, H * 97], BF16, tag=f"V{k}", name=f"Vst{k}") for k in range(KT)]
            cst = [pers.tile([128, QCH], BF16, tag=f"cst{c}", name=f"cst{c}") for c in range(EC)]
            ones = pers.tile([1, 96], BF16, tag="ones", name="ones")

            nc.vector.memset(ones[:], 1.0)

            # input DMAs
            for c in range(EC):
                r = slice(128 * c, 128 * (c + 1))
                nc.sync.dma_start(xt[c][:], xT[r, :])
                nc.sync.dma_start(xtq[c][:], xTq[r, :])
                nc.sync.dma_start(wqs[c][:], wq[r, :])
                nc.sync.dma_start(wks[c][:], wk[r, :])
                nc.sync.dma_start(wvs[c][:], wv[r, :])
                nc.sync.dma_start(wos[c][:], wo[r, :])

            # ---- K / Q projections: Ksb[h] = (wk_h)^T xT, d-major
            for h in range(H):
                hs = slice(96 * h, 96 * h + 96)
                for n in range(S // 512):
                    ps = psp.tile([128, 1024], F32, tag="lps", name="lps")
                    for c in range(EC):
                        nc.tensor.matmul(ps[0:96, 0:512], wks[c][:, hs],
                                         xt[c][:, 512 * n:512 * n + 512],
                                         start=(c == 0), stop=(c == EC - 1))
                    nc.vector.tensor_copy(Ksb[h][:, 512 * n:512 * n + 512],
                                          ps[0:96, 0:512])
                for n in range(QCH // 512):
                    ps = psp.tile([128, 1024], F32, tag="lps", name="lps")
                    for c in range(EC):
                        nc.tensor.matmul(ps[0:96, 0:512], wqs[c][:, hs],
                                         xtq[c][:, 512 * n:512 * n + 512],
                                         start=(c == 0), stop=(c == EC - 1))
                    nc.vector.tensor_copy(Qsb[h][:, 512 * n:512 * n + 512],
                                          ps[0:96, 0:512])

            # ---- V projection, k-major, heads interleaved with ones columns
            for k in range(KT):
                ks = slice(128 * k, 128 * (k + 1))
                ps = psp.tile([128, 1024], F32, tag="lps", name="lps")
                for c in range(EC):
                    nc.tensor.matmul(ps[:, 0:480], xt[c][:, ks],
                                     wvs[c][:, 0:480],
                                     start=(c == 0), stop=(c == EC - 1))
                    nc.tensor.matmul(ps[:, 512:800], xt[c][:, ks],
                                     wvs[c][:, 480:768],
                                     start=(c == 0), stop=(c == EC - 1))
                dst1 = Vst[k][:, 0:485].rearrange("p (h x) -> p h x", x=97)[:, :, 0:96]
                src1 = ps[:, 0:480].rearrange("p (h x) -> p h x", x=96)
                nc.vector.tensor_copy(dst1, src1)
                dst2 = Vst[k][:, 485:776].rearrange("p (h x) -> p h x", x=97)[:, :, 0:96]
                src2 = ps[:, 512:800].rearrange("p (h x) -> p h x", x=96)
                nc.vector.tensor_copy(dst2, src2)
                nc.vector.memset(Vst[k][:, 96:776:97], 1.0)

            # ---- attention per head
            for h in range(H):
                est = stripp.tile([128, 2944], BF16, tag="strip", name="est")
                nc.gpsimd.dma_start(est[:], _strip_src_ap(etre, h))
                ctx = psctxp.tile([97, 1024], F32, tag="ctx", name="ctx")
                vsl = slice(97 * h, 97 * h + 97)
                for k in range(KT):
                    lps = psp.tile([128, 1024], F32, tag="lps", name="lps")
                    ksl = slice(128 * k, 128 * (k + 1))
                    nc.tensor.matmul(lps[:, 0:512], Ksb[h][:, ksl],
                                     Qsb[h][:, 0:512], start=True, stop=True)
                    nc.tensor.matmul(lps[:, 512:1024], Ksb[h][:, ksl],
                                     Qsb[h][:, 512:1024], start=True, stop=True)
                    at = attnp.tile([128, 1024], BF16, tag="attn", name="at")
                    nc.scalar.activation(at[:], lps[:], Exp)
                    g0 = 128 * k
                    nc.vector.tensor_mul(at[:], at[:], est[:, g0:g0 + 1024])
                    nc.tensor.matmul(ctx[:, 0:512], Vst[k][:, vsl],
                                     at[:, 0:512],
                                     start=(k == 0), stop=(k == KT - 1))
                    nc.tensor.matmul(ctx[:, 512:1024], Vst[k][:, vsl],
                                     at[:, 512:1024],
                                     start=(k == 0), stop=(k == KT - 1))
                # normalize: rows 0..96 of ctx divided by row 96 (the sum)
                rec32 = smallp.tile([1, 1024], F32, tag="rec32", name="rec32")
                nc.vector.reciprocal(rec32[:], ctx[96:97, :])
                rec = smallp.tile([1, 1024], BF16, tag="rec", name="rec")
                nc.vector.tensor_copy(rec[:], rec32[:])
                bc = psp.tile([128, 1024], F32, tag="lps", name="lps")
                nc.tensor.matmul(bc[0:96, 0:512], ones[:], rec[:, 0:512],
                                 start=True, stop=True)
                nc.tensor.matmul(bc[0:96, 512:1024], ones[:], rec[:, 512:1024],
                                 start=True, stop=True)
                bcs = smallp.tile([96, 1024], BF16, tag="bcs", name="bcs")
                nc.scalar.copy(bcs[:], bc[0:96, :])
                # scatter into the stacked-ctx tiles in 32-partition segments
                # (SBUF engine APs may only start at partition 0/32/64/96,
                # with span caps of 128/32/64/32 respectively)
                for seg in range(3):
                    lo = 96 * h + 32 * seg
                    ti, po, off = lo // 128, lo % 128, 32 * seg
                    nc.vector.tensor_mul(cst[ti][po:po + 32, :],
                                         ctx[off:off + 32, :],
                                         bcs[off:off + 32, :])

            # ---- output projection
            for qt in range(QCH // 128):
                qs = slice(128 * qt, 128 * (qt + 1))
                op = psp.tile([128, 1024], F32, tag="lps", name="lps")
                for c in range(EC):
                    nc.tensor.matmul(op[:, 0:512], cst[c][:, qs],
                                     wos[c][:, 0:512],
                                     start=(c == 0), stop=(c == EC - 1))
                    nc.tensor.matmul(op[:, 512:768], cst[c][:, qs],
                                     wos[c][:, 512:768],
                                     start=(c == 0), stop=(c == EC - 1))
                osb = osbp.tile([128, E], F32, tag="osb", name="osb")
                nc.vector.tensor_copy(osb[:, 0:512], op[:, 0:512])
                nc.scalar.copy(osb[:, 512:768], op[:, 512:768])
                nc.sync.dma_start(out[qs, :], osb[:])

    nc.compile()
    return nc


def get_nc():
    if "nc" not in _NC_CACHE:
        _NC_CACHE["nc"] = build_nc()
    return _NC_CACHE["nc"]


def host_prep(x, w_q, w_k, w_v, w_o, rel_emb):
    x = np.asarray(x, np.float32)
    w_q = np.asarray(w_q, np.float32)
    w_k = np.asarray(w_k, np.float32)
    w_v = np.asarray(w_v, np.float32)
    w_o = np.asarray(w_o, np.float32)
    rel = np.asarray(rel_emb, np.float32)

    scale = D ** -0.5
    wq = (w_q * scale).transpose(1, 0, 2).reshape(E, H * D).astype(NPBF16)
    wk = w_k.transpose(1, 0, 2).reshape(E, H * D).astype(NPBF16)
    wv = w_v.transpose(1, 0, 2).reshape(E, H * D).astype(NPBF16)
    wo = w_o.reshape(H * D, E).astype(NPBF16)

    emb_sum = rel[..., 0].sum(-1)                                 # (h, 199)
    kk = np.arange(2 * S - 1)
    t = emb_sum[:, np.clip(kk - (S - 1), -(MAX_DIST - 1), MAX_DIST - 1)
                + MAX_DIST - 1]                                   # (h, 4095)
    et = np.exp(t)                                                # exp(bias) values

    j = np.arange(3072)
    in_maps = []
    for core in range(NCORES):
        b, q0 = core // 2, (core % 2) * QCH
        # key axis REVERSED for K/V (makes the bias strip DMA all-positive)
        xT = np.ascontiguousarray(x[b].T[:, ::-1]).astype(NPBF16)  # (768, 2048)
        xTq = np.ascontiguousarray(x[b].T[:, q0:q0 + QCH]).astype(NPBF16)
        etre = et[:, np.clip(2 * S - 2 - q0 - j, 0, 2 * S - 2)].astype(NPBF16)
        in_maps.append(dict(xT=xT, xTq=xTq, wq=wq, wk=wk, wv=wv, wo=wo,
                            etre=etre))
    return in_maps


def run(inputs, trace=False, trace_cores=None):
    nc = get_nc()
    in_maps = host_prep(**inputs)
    res = bass_utils.run_bass_kernel_spmd(
        nc, in_maps, core_ids=list(range(NCORES)),
        trace=trace, trace_cores=trace_cores)
    out = np.zeros((B, S, E), np.float32)
    for core in range(NCORES):
        b, q0 = core // 2, (core % 2) * QCH
        out[b, q0:q0 + QCH, :] = res.results[core]["out"]
    return out, res


def kernel(x, w_q, w_k, w_v, w_o, rel_emb):
    out, _ = run(dict(x=x, w_q=w_q, w_k=w_k, w_v=w_v, w_o=w_o,
                      rel_emb=rel_emb))
    return out


# revision 11
# speedup vs baseline: 1.0789x; 1.0789x over previous
"""Trainium2 Bass kernel for nn_GaddyMultiheadAttention.

Full-input contract: kernel(**inputs) takes the unsharded tensors
  x (4,2048,768) f32, w_q/w_k/w_v (8,768,96), w_o (8,96,768),
  rel_emb (8,199,96,1)
and returns the full (4,2048,768) f32 output.

Sharding: 8 cores = 4 batches x 2 query-chunks of 1024. No collectives.

Per-core kernel (SPMD, one program):
  - host prep: xT = x[b].T in bf16; weights reshaped to (e, h*d)/(h*d, e)
    bf16 (w_q pre-scaled by d^-0.5); relative-position bias Toeplitz values
    exp(t) reversed+shifted per core ("etre", (8, 3072) bf16).
  - projections: K_h (d,k) d-major, Q_h (d,q) d-major, V (k, h*d interleaved
    with a ones column per head) k-major; all via bf16 matmuls.
  - attention per head, transposed-logits layout: logits_T (128k x 1024q)
    = K_tile^T Q in PSUM, exp on ScalarE, multiply by the exp(bias) strip
    (diagonal sliding-window DMA of etre) on VectorE, then ctx_T (97 x 1024)
    += [V_tile | 1]^T attn on TensorE; ones column yields the softmax
    denominator; normalize via reciprocal + ones-outer-product broadcast.
  - output projection from the stacked normalized ctx (h*d x q) with
    wo (h*d, e), DMA out (1024, 768) f32.
"""

import numpy as np
import ml_dtypes

import concourse.bacc as bacc
import concourse.mybir as mybir
import concourse.tile as tile
from concourse import bass_utils
from bass_rust import VecI64Pair

B, S, E, H, D = 4, 2048, 768, 8, 96
MAX_DIST = 100
QCH = 1024                      # queries per core
NCORES = 8
EC = E // 128                   # 6 contraction chunks of 128
KT = S // 128                   # 16 key tiles
BF16 = mybir.dt.bfloat16
F32 = mybir.dt.float32
NPBF16 = ml_dtypes.bfloat16

_NC_CACHE = {}


def _strip_src_ap(etre_ap, h):
    """AP reading etre[h, p + g] for p in [0,128), g in [0,2944).

    Keys are processed in REVERSED sequence order (host reverses the key
    axis of xT, so K/V columns/rows are key 2047-j').  Then for key-tile kt,
    strip row p = key 2047-(128*kt+p), and
    exp(bias)[key, query=q0+f] == strip[p, 128*kt + f].
    All-positive unit steps — walrus rejects negative partition steps.
    """
    a = etre_ap.copy()
    a.offset = a.offset + 3072 * h
    a.ap = VecI64Pair([[1, 128], [1, 2944]])
    return a


def build_nc():
    nc = bacc.Bacc("TRN2", target_bir_lowering=False, debug=False,
                   num_devices=NCORES)
    xT = nc.dram_tensor("xT", (E, S), BF16, kind="ExternalInput").ap()
    xTq = nc.dram_tensor("xTq", (E, QCH), BF16, kind="ExternalInput").ap()
    wq = nc.dram_tensor("wq", (E, H * D), BF16, kind="ExternalInput").ap()
    wk = nc.dram_tensor("wk", (E, H * D), BF16, kind="ExternalInput").ap()
    wv = nc.dram_tensor("wv", (E, H * D), BF16, kind="ExternalInput").ap()
    wo = nc.dram_tensor("wo", (H * D, E), BF16, kind="ExternalInput").ap()
    etre = nc.dram_tensor("etre", (H, 3072), BF16, kind="ExternalInput").ap()
    out = nc.dram_tensor("out", (QCH, E), F32, kind="ExternalOutput").ap()

    Exp = mybir.ActivationFunctionType.Exp
    Ln = mybir.ActivationFunctionType.Ln

    with tile.TileContext(nc) as tc:
        with (
            tc.tile_pool(name="pers", bufs=1) as pers,
            tc.tile_pool(name="stripp", bufs=2) as stripp,
            tc.tile_pool(name="attnp", bufs=4) as attnp,
            tc.tile_pool(name="smallp", bufs=2) as smallp,
            tc.tile_pool(name="osbp", bufs=2) as osbp,
            tc.tile_pool(name="psp", bufs=2, space="PSUM") as psp,
            tc.tile_pool(name="psctxp", bufs=2, space="PSUM") as psctxp,
        ):
            # persistent SBUF tiles
            xt = [pers.tile([128, S], BF16, tag=f"xt{c}", name=f"xt{c}") for c in range(EC)]
            xtq = [pers.tile([128, QCH], BF16, tag=f"xtq{c}", name=f"xtq{c}") for c in range(EC)]
            wqs = [pers.tile([128, H * D], BF16, tag=f"wq{c}", name=f"wqs{c}") for c in range(EC)]
            wks = [pers.tile([128, H * D], BF16, tag=f"wk{c}", name=f"wks{c}") for c in range(EC)]
            wvs = [pers.tile([128, H * D], BF16, tag=f"wv{c}", name=f"wvs{c}") for c in range(EC)]
            wos = [pers.tile([128, E], BF16, tag=f"wo{c}", name=f"wos{c}") for c in range(EC)]
            Ksb = [pers.tile([96, S], BF16, tag=f"K{h}", name=f"Ksb{h}") for h in range(H)]
            Qsb = [pers.tile([96, QCH], BF16, tag=f"Q{h}", name=f"Qsb{h}") for h in range(H)]
            Vst = [pers.tile([128, H * 97], BF16, tag=f"V{k}", name=f"Vst{k}") for k in range(KT)]
            cst = [pers.tile([128, QCH], BF16, tag=f"cst{c}", name=f"cst{c}") for c in range(EC)]
            ones = pers.tile([1, 96], BF16, tag="ones", name="ones")

            nc.vector.memset(ones[:], 1.0)

            # input DMAs
            for c in range(EC):
                r = slice(128 * c, 128 * (c + 1))
                nc.sync.dma_start(xt[c][:], xT[r, :])
                nc.sync.dma_start(xtq[c][:], xTq[r, :])
                nc.sync.dma_start(wqs[c][:], wq[r, :])
                nc.sync.dma_start(wks[c][:], wk[r, :])
                nc.sync.dma_start(wvs[c][:], wv[r, :])
                nc.sync.dma_start(wos[c][:], wo[r, :])

            # ---- K / Q projections: Ksb[h] = (wk_h)^T xT, d-major
            for h in range(H):
                hs = slice(96 * h, 96 * h + 96)
                for n in range(S // 512):
                    ps = psp.tile([128, 1024], F32, tag="lps", name="lps")
                    for c in range(EC):
                        nc.tensor.matmul(ps[0:96, 0:512], wks[c][:, hs],
                                         xt[c][:, 512 * n:512 * n + 512],
                                         start=(c == 0), stop=(c == EC - 1))
                    nc.vector.tensor_copy(Ksb[h][:, 512 * n:512 * n + 512],
                                          ps[0:96, 0:512])
                for n in range(QCH // 512):
                    ps = psp.tile([128, 1024], F32, tag="lps", name="lps")
                    for c in range(EC):
                        nc.tensor.matmul(ps[0:96, 0:512], wqs[c][:, hs],
                                         xtq[c][:, 512 * n:512 * n + 512],
                                         start=(c == 0), stop=(c == EC - 1))
                    nc.vector.tensor_copy(Qsb[h][:, 512 * n:512 * n + 512],
                                          ps[0:96, 0:512])

            # ---- V projection, k-major, heads interleaved with ones columns
            for k in range(KT):
                ks = slice(128 * k, 128 * (k + 1))
                ps = psp.tile([128, 1024], F32, tag="lps", name="lps")
                for c in range(EC):
                    nc.tensor.matmul(ps[:, 0:480], xt[c][:, ks],
                                     wvs[c][:, 0:480],
                                     start=(c == 0), stop=(c == EC - 1))
                    nc.tensor.matmul(ps[:, 512:800], xt[c][:, ks],
                                     wvs[c][:, 480:768],
                                     start=(c == 0), stop=(c == EC - 1))
                dst1 = Vst[k][:, 0:485].rearrange("p (h x) -> p h x", x=97)[:, :, 0:96]
                src1 = ps[:, 0:480].rearrange("p (h x) -> p h x", x=96)
                nc.vector.tensor_copy(dst1, src1)
                dst2 = Vst[k][:, 485:776].rearrange("p (h x) -> p h x", x=97)[:, :, 0:96]
                src2 = ps[:, 512:800].rearrange("p (h x) -> p h x", x=96)
                nc.vector.tensor_copy(dst2, src2)
                nc.vector.memset(Vst[k][:, 96:776:97], 1.0)

            # ---- attention per head
            for h in range(H):
                est = stripp.tile([128, 2944], BF16, tag="strip", name="est")
                nc.gpsimd.dma_start(est[:], _strip_src_ap(etre, h))
                ctx = psctxp.tile([97, 1024], F32, tag="ctx", name="ctx")
                vsl = slice(97 * h, 97 * h + 97)
                for k in range(KT):
                    lps = psp.tile([128, 1024], F32, tag="lps", name="lps")
                    ksl = slice(128 * k, 128 * (k + 1))
                    nc.tensor.matmul(lps[:, 0:512], Ksb[h][:, ksl],
                                     Qsb[h][:, 0:512], start=True, stop=True)
                    nc.tensor.matmul(lps[:, 512:1024], Ksb[h][:, ksl],
                                     Qsb[h][:, 512:1024], start=True, stop=True)
                    at = attnp.tile([128, 1024], BF16, tag="attn", name="at")
                    nc.scalar.activation(at[:], lps[:], Exp)
                    g0 = 128 * k
                    nc.vector.tensor_mul(at[:], at[:], est[:, g0:g0 + 1024])
                    nc.tensor.matmul(ctx[:, 0:512], Vst[k][:, vsl],
                                     at[:, 0:512],
                                     start=(k == 0), stop=(k == KT - 1))
                    nc.tensor.matmul(ctx[:, 512:1024], Vst[k][:, vsl],
                                     at[:, 512:1024],
                                     start=(k == 0), stop=(k == KT - 1))
                # normalize: rows 0..96 of ctx divided by row 96 (the sum)
                # 1/S via exp(-ln(S)) on ScalarE: a single-partition DVE
                # reciprocal measures ~6.6us; two ACT passes are ~2.3us and
                # Exp/Ln share one ACT table set.
                ln32 = smallp.tile([1, 1024], F32, tag="ln32", name="ln32")
                nc.scalar.activation(ln32[:], ctx[96:97, :], Ln)
                rec = smallp.tile([1, 1024], BF16, tag="rec", name="rec")
                nc.scalar.activation(rec[:], ln32[:], Exp, scale=-1.0)
                bc = psp.tile([128, 1024], F32, tag="lps", name="lps")
                nc.tensor.matmul(bc[0:96, 0:512], ones[:], rec[:, 0:512],
                                 start=True, stop=True)
                nc.tensor.matmul(bc[0:96, 512:1024], ones[:], rec[:, 512:1024],
                                 start=True, stop=True)
                bcs = smallp.tile([96, 1024], BF16, tag="bcs", name="bcs")
                nc.vector.tensor_copy(bcs[:], bc[0:96, :])
                # scatter into the stacked-ctx tiles in 32-partition segments
                # (SBUF engine APs may only start at partition 0/32/64/96,
                # with span caps of 128/32/64/32 respectively)
                for seg in range(3):
                    lo = 96 * h + 32 * seg
                    ti, po, off = lo // 128, lo % 128, 32 * seg
                    nc.vector.tensor_mul(cst[ti][po:po + 32, :],
                                         ctx[off:off + 32, :],
                                         bcs[off:off + 32, :])

            # ---- output projection
            for qt in range(QCH // 128):
                qs = slice(128 * qt, 128 * (qt + 1))
                op = psp.tile([128, 1024], F32, tag="lps", name="lps")
                for c in range(EC):
                    nc.tensor.matmul(op[:, 0:512], cst[c][:, qs],
                                     wos[c][:, 0:512],
                                     start=(c == 0), stop=(c == EC - 1))
                    nc.tensor.matmul(op[:, 512:768], cst[c][:, qs],
                                     wos[c][:, 512:768],
                                     start=(c == 0), stop=(c == EC - 1))
                osb = osbp.tile([128, E], F32, tag="osb", name="osb")
                nc.vector.tensor_copy(osb[:, 0:512], op[:, 0:512])
                nc.scalar.copy(osb[:, 512:768], op[:, 512:768])
                nc.sync.dma_start(out[qs, :], osb[:])

    nc.compile()
    return nc


def get_nc():
    if "nc" not in _NC_CACHE:
        _NC_CACHE["nc"] = build_nc()
    return _NC_CACHE["nc"]


def host_prep(x, w_q, w_k, w_v, w_o, rel_emb):
    x = np.asarray(x, np.float32)
    w_q = np.asarray(w_q, np.float32)
    w_k = np.asarray(w_k, np.float32)
    w_v = np.asarray(w_v, np.float32)
    w_o = np.asarray(w_o, np.float32)
    rel = np.asarray(rel_emb, np.float32)

    scale = D ** -0.5
    wq = (w_q * scale).transpose(1, 0, 2).reshape(E, H * D).astype(NPBF16)
    wk = w_k.transpose(1, 0, 2).reshape(E, H * D).astype(NPBF16)
    wv = w_v.transpose(1, 0, 2).reshape(E, H * D).astype(NPBF16)
    wo = w_o.reshape(H * D, E).astype(NPBF16)

    emb_sum = rel[..., 0].sum(-1)                                 # (h, 199)
    kk = np.arange(2 * S - 1)
    t = emb_sum[:, np.clip(kk - (S - 1), -(MAX_DIST - 1), MAX_DIST - 1)
                + MAX_DIST - 1]                                   # (h, 4095)
    et = np.exp(t)                                                # exp(bias) values

    j = np.arange(3072)
    in_maps = []
    for core in range(NCORES):
        b, q0 = core // 2, (core % 2) * QCH
        # key axis REVERSED for K/V (makes the bias strip DMA all-positive)
        xT = np.ascontiguousarray(x[b].T[:, ::-1]).astype(NPBF16)  # (768, 2048)
        xTq = np.ascontiguousarray(x[b].T[:, q0:q0 + QCH]).astype(NPBF16)
        etre = et[:, np.clip(2 * S - 2 - q0 - j, 0, 2 * S - 2)].astype(NPBF16)
        in_maps.append(dict(xT=xT, xTq=xTq, wq=wq, wk=wk, wv=wv, wo=wo,
                            etre=etre))
    return in_maps


def run(inputs, trace=False, trace_cores=None):
    nc = get_nc()
    in_maps = host_prep(**inputs)
    res = bass_utils.run_bass_kernel_spmd(
        nc, in_maps, core_ids=list(range(NCORES)),
        trace=trace, trace_cores=trace_cores)
    out = np.zeros((B, S, E), np.float32)
    for core in range(NCORES):
        b, q0 = core // 2, (core % 2) * QCH
        out[b, q0:q0 + QCH, :] = res.results[core]["out"]
    return out, res


def kernel(x, w_q, w_k, w_v, w_o, rel_emb):
    out, _ = run(dict(x=x, w_q=w_q, w_k=w_k, w_v=w_v, w_o=w_o,
                      rel_emb=rel_emb))
    return out


# revision 17
# speedup vs baseline: 1.1077x; 1.0267x over previous
"""Trainium2 Bass kernel for nn_GaddyMultiheadAttention.

Full-input contract: kernel(**inputs) takes the unsharded tensors
  x (4,2048,768) f32, w_q/w_k/w_v (8,768,96), w_o (8,96,768),
  rel_emb (8,199,96,1)
and returns the full (4,2048,768) f32 output.

Sharding: 8 cores = 4 batches x 2 query-chunks of 1024. No collectives.

Per-core kernel (SPMD, one program):
  - host prep: xT = x[b].T in bf16; weights reshaped to (e, h*d)/(h*d, e)
    bf16 (w_q pre-scaled by d^-0.5); relative-position bias Toeplitz values
    exp(t) reversed+shifted per core ("etre", (8, 3072) bf16).
  - projections: K_h (d,k) d-major, Q_h (d,q) d-major, V (k, h*d interleaved
    with a ones column per head) k-major; all via bf16 matmuls.
  - attention per head, transposed-logits layout: logits_T (128k x 1024q)
    = K_tile^T Q in PSUM, exp on ScalarE, multiply by the exp(bias) strip
    (diagonal sliding-window DMA of etre) on VectorE, then ctx_T (97 x 1024)
    += [V_tile | 1]^T attn on TensorE; ones column yields the softmax
    denominator; normalize via reciprocal + ones-outer-product broadcast.
  - output projection from the stacked normalized ctx (h*d x q) with
    wo (h*d, e), DMA out (1024, 768) f32.
"""

import numpy as np
import ml_dtypes

import concourse.bacc as bacc
import concourse.mybir as mybir
import concourse.tile as tile
from concourse import bass_utils
from bass_rust import VecI64Pair

B, S, E, H, D = 4, 2048, 768, 8, 96
MAX_DIST = 100
QCH = 1024                      # queries per core
NCORES = 8
EC = E // 128                   # 6 contraction chunks of 128
KT = S // 128                   # 16 key tiles
BF16 = mybir.dt.bfloat16
F32 = mybir.dt.float32
NPBF16 = ml_dtypes.bfloat16

_NC_CACHE = {}


def _strip_src_ap(etre_ap, h):
    """AP reading etre[h, p + g] for p in [0,128), g in [0,2944).

    Keys are processed in REVERSED sequence order (host reverses the key
    axis of xT, so K/V columns/rows are key 2047-j').  Then for key-tile kt,
    strip row p = key 2047-(128*kt+p), and
    exp(bias)[key, query=q0+f] == strip[p, 128*kt + f].
    All-positive unit steps — walrus rejects negative partition steps.
    """
    a = etre_ap.copy()
    a.offset = a.offset + 3072 * h
    a.ap = VecI64Pair([[1, 128], [1, 2944]])
    return a


def build_nc():
    nc = bacc.Bacc("TRN2", target_bir_lowering=False, debug=False,
                   num_devices=NCORES)
    xT = nc.dram_tensor("xT", (E, S), BF16, kind="ExternalInput").ap()
    xTq = nc.dram_tensor("xTq", (E, QCH), BF16, kind="ExternalInput").ap()
    wq = nc.dram_tensor("wq", (E, H * D), BF16, kind="ExternalInput").ap()
    wk = nc.dram_tensor("wk", (E, H * D), BF16, kind="ExternalInput").ap()
    wv = nc.dram_tensor("wv", (E, H * D), BF16, kind="ExternalInput").ap()
    wo = nc.dram_tensor("wo", (H * D, E), BF16, kind="ExternalInput").ap()
    etre = nc.dram_tensor("etre", (H, 3072), BF16, kind="ExternalInput").ap()
    out = nc.dram_tensor("out", (QCH, E), F32, kind="ExternalOutput").ap()

    Exp = mybir.ActivationFunctionType.Exp
    Ln = mybir.ActivationFunctionType.Ln

    with tile.TileContext(nc) as tc:
        with (
            tc.tile_pool(name="pers", bufs=1) as pers,
            tc.tile_pool(name="stripp", bufs=2) as stripp,
            tc.tile_pool(name="attnp", bufs=4) as attnp,
            tc.tile_pool(name="smallp", bufs=2) as smallp,
            tc.tile_pool(name="osbp", bufs=2) as osbp,
            tc.tile_pool(name="psp", bufs=2, space="PSUM") as psp,
            tc.tile_pool(name="psctxp", bufs=2, space="PSUM") as psctxp,
        ):
            # persistent SBUF tiles
            xt = [pers.tile([128, S], BF16, tag=f"xt{c}", name=f"xt{c}") for c in range(EC)]
            xtq = [pers.tile([128, QCH], BF16, tag=f"xtq{c}", name=f"xtq{c}") for c in range(EC)]
            wqs = [pers.tile([128, H * D], BF16, tag=f"wq{c}", name=f"wqs{c}") for c in range(EC)]
            wks = [pers.tile([128, H * D], BF16, tag=f"wk{c}", name=f"wks{c}") for c in range(EC)]
            wvs = [pers.tile([128, H * D], BF16, tag=f"wv{c}", name=f"wvs{c}") for c in range(EC)]
            wos = [pers.tile([128, E], BF16, tag=f"wo{c}", name=f"wos{c}") for c in range(EC)]
            Ksb = [pers.tile([96, S], BF16, tag=f"K{h}", name=f"Ksb{h}") for h in range(H)]
            Qsb = [pers.tile([96, QCH], BF16, tag=f"Q{h}", name=f"Qsb{h}") for h in range(H)]
            Vst = [pers.tile([128, H * 97], BF16, tag=f"V{k}", name=f"Vst{k}") for k in range(KT)]
            cst = [pers.tile([128, QCH], BF16, tag=f"cst{c}", name=f"cst{c}") for c in range(EC)]
            ones = pers.tile([1, 96], BF16, tag="ones", name="ones")
            ones2 = pers.tile([1, 1024], BF16, tag="ones2", name="ones2")

            nc.vector.memset(ones[:], 1.0)
            nc.vector.memset(ones2[:], 1.0)

            # input DMAs — wk+xt first so the K projection starts ASAP
            for c in range(EC):
                r = slice(128 * c, 128 * (c + 1))
                nc.sync.dma_start(wks[c][:], wk[r, :])
                nc.sync.dma_start(xt[c][:], xT[r, :])
            for c in range(EC):
                r = slice(128 * c, 128 * (c + 1))
                nc.sync.dma_start(xtq[c][:], xTq[r, :])
                nc.sync.dma_start(wqs[c][:], wq[r, :])
                nc.sync.dma_start(wvs[c][:], wv[r, :])
                nc.sync.dma_start(wos[c][:], wo[r, :])

            # ---- K / Q projections: Ksb[h] = (wk_h)^T xT, d-major
            for h in range(H):
                hs = slice(96 * h, 96 * h + 96)
                for n in range(S // 512):
                    ps = psp.tile([128, 1024], F32, tag="lps", name="lps")
                    for c in range(EC):
                        nc.tensor.matmul(ps[0:96, 0:512], wks[c][:, hs],
                                         xt[c][:, 512 * n:512 * n + 512],
                                         start=(c == 0), stop=(c == EC - 1))
                    nc.vector.tensor_copy(Ksb[h][:, 512 * n:512 * n + 512],
                                          ps[0:96, 0:512])
                for n in range(QCH // 512):
                    ps = psp.tile([128, 1024], F32, tag="lps", name="lps")
                    for c in range(EC):
                        nc.tensor.matmul(ps[0:96, 0:512], wqs[c][:, hs],
                                         xtq[c][:, 512 * n:512 * n + 512],
                                         start=(c == 0), stop=(c == EC - 1))
                    nc.vector.tensor_copy(Qsb[h][:, 512 * n:512 * n + 512],
                                          ps[0:96, 0:512])

            # ---- V projection, k-major, heads interleaved with ones columns
            for k in range(KT):
                ks = slice(128 * k, 128 * (k + 1))
                ps = psp.tile([128, 1024], F32, tag="lps", name="lps")
                for c in range(EC):
                    nc.tensor.matmul(ps[:, 0:480], xt[c][:, ks],
                                     wvs[c][:, 0:480],
                                     start=(c == 0), stop=(c == EC - 1))
                    nc.tensor.matmul(ps[:, 512:800], xt[c][:, ks],
                                     wvs[c][:, 480:768],
                                     start=(c == 0), stop=(c == EC - 1))
                dst1 = Vst[k][:, 0:485].rearrange("p (h x) -> p h x", x=97)[:, :, 0:96]
                src1 = ps[:, 0:480].rearrange("p (h x) -> p h x", x=96)
                nc.vector.tensor_copy(dst1, src1)
                dst2 = Vst[k][:, 485:776].rearrange("p (h x) -> p h x", x=97)[:, :, 0:96]
                src2 = ps[:, 512:800].rearrange("p (h x) -> p h x", x=96)
                nc.vector.tensor_copy(dst2, src2)
                nc.vector.memset(Vst[k][:, 96:776:97], 1.0)

            # ---- attention per head, with normalization of head h-1
            # software-pipelined into head h's emission (keeps the per-head
            # normalize chain off TensorE's critical path)
            div = mybir.AluOpType.divide

            def emit_norm_recip_q(st, ctx, quarter):
                # one quarter of the 1/S reciprocal (spreads the ~6.5us
                # single-partition DVE reciprocal into 4 short ops)
                if quarter == 0:
                    st["rec32"] = smallp.tile([1, 1024], F32, tag="rec32",
                                              name="rec32")
                qs = slice(256 * quarter, 256 * (quarter + 1))
                nc.vector.reciprocal(st["rec32"][0:1, qs], ctx[96:97, qs])

            def emit_norm_pre(h, ctx, st):
                # cast 1/S to bf16 and broadcast across 96 partitions (C=1
                # matmul against a ones row)
                rec = smallp.tile([1, 1024], BF16, tag="rec", name="rec")
                nc.vector.tensor_copy(rec[:], st["rec32"][:])
                bc = psp.tile([128, 1024], F32, tag="lps", name="lps")
                nc.tensor.matmul(bc[0:96, 0:512], ones[:], rec[:, 0:512],
                                 start=True, stop=True)
                nc.tensor.matmul(bc[0:96, 512:1024], ones[:], rec[:, 512:1024],
                                 start=True, stop=True)
                bcs = smallp.tile([96, 1024], BF16, tag="bcs", name="bcs")
                nc.vector.tensor_copy(bcs[:], bc[0:96, :])
                return bcs

            def emit_norm_div(h, ctx, bcs, seg):
                # cst piece = ctx * (1/S), in 32-partition segments (SBUF
                # engine APs may only start at partition 0/32/64/96)
                lo = 96 * h + 32 * seg
                ti, po, off = lo // 128, lo % 128, 32 * seg
                nc.vector.tensor_mul(cst[ti][po:po + 32, :],
                                     ctx[off:off + 32, :],
                                     bcs[off:off + 32, :])

            pending = None  # (h, ctx) awaiting normalization
            norm_state = {}
            for h in range(H):
                est = stripp.tile([128, 2944], BF16, tag="strip", name="est")
                nc.gpsimd.dma_start(est[:], _strip_src_ap(etre, h))
                ctx = psctxp.tile([97, 1024], F32, tag="ctx", name="ctx")
                vsl = slice(97 * h, 97 * h + 97)
                for k in range(KT):
                    lps = psp.tile([128, 1024], F32, tag="lps", name="lps")
                    ksl = slice(128 * k, 128 * (k + 1))
                    nc.tensor.matmul(lps[:, 0:512], Ksb[h][:, ksl],
                                     Qsb[h][:, 0:512], start=True, stop=True)
                    nc.tensor.matmul(lps[:, 512:1024], Ksb[h][:, ksl],
                                     Qsb[h][:, 512:1024], start=True, stop=True)
                    at = attnp.tile([128, 1024], BF16, tag="attn", name="at")
                    nc.scalar.activation(at[:], lps[:], Exp)
                    g0 = 128 * k
                    nc.vector.tensor_mul(at[:], at[:], est[:, g0:g0 + 1024])
                    nc.tensor.matmul(ctx[:, 0:512], Vst[k][:, vsl],
                                     at[:, 0:512],
                                     start=(k == 0), stop=(k == KT - 1))
                    nc.tensor.matmul(ctx[:, 512:1024], Vst[k][:, vsl],
                                     at[:, 512:1024],
                                     start=(k == 0), stop=(k == KT - 1))
                    # pipelined normalize of the previous head
                    if pending is not None:
                        ph, pctx = pending
                        if k in (1, 3, 5, 7):
                            emit_norm_recip_q(norm_state, pctx, (k - 1) // 2)
                        elif k == 8:
                            norm_state["bcs"] = emit_norm_pre(ph, pctx,
                                                              norm_state)
                        elif k in (10, 12, 14):
                            emit_norm_div(ph, pctx, norm_state["bcs"],
                                          {10: 0, 12: 1, 14: 2}[k])
                            if k == 14:
                                pending = None
                pending = (h, ctx)
            ph, pctx = pending
            st = {}
            for q in range(4):
                emit_norm_recip_q(st, pctx, q)
            bcs = emit_norm_pre(ph, pctx, st)
            for seg in range(3):
                emit_norm_div(ph, pctx, bcs, seg)

            # ---- output projection
            for qt in range(QCH // 128):
                qs = slice(128 * qt, 128 * (qt + 1))
                op = psp.tile([128, 1024], F32, tag="lps", name="lps")
                for c in range(EC):
                    nc.tensor.matmul(op[:, 0:512], cst[c][:, qs],
                                     wos[c][:, 0:512],
                                     start=(c == 0), stop=(c == EC - 1))
                    nc.tensor.matmul(op[:, 512:768], cst[c][:, qs],
                                     wos[c][:, 512:768],
                                     start=(c == 0), stop=(c == EC - 1))
                osb = osbp.tile([128, E], F32, tag="osb", name="osb")
                nc.vector.tensor_copy(osb[:, 0:512], op[:, 0:512])
                nc.scalar.copy(osb[:, 512:768], op[:, 512:768])
                nc.sync.dma_start(out[qs, :], osb[:])

    nc.compile()
    return nc


def get_nc():
    if "nc" not in _NC_CACHE:
        _NC_CACHE["nc"] = build_nc()
    return _NC_CACHE["nc"]


def host_prep(x, w_q, w_k, w_v, w_o, rel_emb):
    x = np.asarray(x, np.float32)
    w_q = np.asarray(w_q, np.float32)
    w_k = np.asarray(w_k, np.float32)
    w_v = np.asarray(w_v, np.float32)
    w_o = np.asarray(w_o, np.float32)
    rel = np.asarray(rel_emb, np.float32)

    scale = D ** -0.5
    wq = (w_q * scale).transpose(1, 0, 2).reshape(E, H * D).astype(NPBF16)
    wk = w_k.transpose(1, 0, 2).reshape(E, H * D).astype(NPBF16)
    wv = w_v.transpose(1, 0, 2).reshape(E, H * D).astype(NPBF16)
    wo = w_o.reshape(H * D, E).astype(NPBF16)

    emb_sum = rel[..., 0].sum(-1)                                 # (h, 199)
    kk = np.arange(2 * S - 1)
    t = emb_sum[:, np.clip(kk - (S - 1), -(MAX_DIST - 1), MAX_DIST - 1)
                + MAX_DIST - 1]                                   # (h, 4095)
    et = np.exp(t)                                                # exp(bias) values

    j = np.arange(3072)
    in_maps = []
    for core in range(NCORES):
        b, q0 = core // 2, (core % 2) * QCH
        # key axis REVERSED for K/V (makes the bias strip DMA all-positive)
        xT = np.ascontiguousarray(x[b].T[:, ::-1]).astype(NPBF16)  # (768, 2048)
        xTq = np.ascontiguousarray(x[b].T[:, q0:q0 + QCH]).astype(NPBF16)
        etre = et[:, np.clip(2 * S - 2 - q0 - j, 0, 2 * S - 2)].astype(NPBF16)
        in_maps.append(dict(xT=xT, xTq=xTq, wq=wq, wk=wk, wv=wv, wo=wo,
                            etre=etre))
    return in_maps


def run(inputs, trace=False, trace_cores=None):
    nc = get_nc()
    in_maps = host_prep(**inputs)
    res = bass_utils.run_bass_kernel_spmd(
        nc, in_maps, core_ids=list(range(NCORES)),
        trace=trace, trace_cores=trace_cores)
    out = np.zeros((B, S, E), np.float32)
    for core in range(NCORES):
        b, q0 = core // 2, (core % 2) * QCH
        out[b, q0:q0 + QCH, :] = res.results[core]["out"]
    return out, res


def kernel(x, w_q, w_k, w_v, w_o, rel_emb):
    out, _ = run(dict(x=x, w_q=w_q, w_k=w_k, w_v=w_v, w_o=w_o,
                      rel_emb=rel_emb))
    return out
